# revision 24
# baseline (speedup 1.0000x reference)
"""Trainium2 Bass kernel for nn_PolicyCausalTransformer_66520453480556.

Sharding: 8 cores = (batch 2) x (sequence quarters 4). Each core owns 32
contiguous steps (384 tokens) of one batch element. Per backbone layer the
K/V activations are AllGathered within each 4-core batch group; everything
else (embedding, LN, QKV/Wo/MLP matmuls, decoder) is local to a core.

Layouts: activations live transposed as X^T [D on partitions, tokens on the
free dim]; weights are DMAd in column-block form [in_k, 128, 128] so only a
few KB/partition of SBUF is resident per matmul phase. Matmul operands are
bf16 (fp32 PSUM accumulation); the residual stream, LN statistics and the
softmax stay in fp32.
"""

import sys

sys.path.insert(0, "/opt/trn_rl_repo")

from contextlib import ExitStack

import numpy as np
import ml_dtypes

import bass_rust
import concourse.bass as bass
import concourse.mybir as mybir
import concourse.tile as tile
import concourse.tile_utils as tile_utils
from concourse.bass_utils import run_bass_kernel_spmd
from concourse.tile import TileContext
from concourse.vector_clock import ScopedClock

# ---------------- model constants ----------------
D = 1024
H = 16
HD = 64
L = 4
T = 128
B = 2
N_IMG = 8
N_THINK = 1
N_ACT = 2
ONE = 12
TTA = 9
S = ONE * T          # 1536
RAM = 2048
CH = 256
LD = 2
DH = 8
DHD = 128

N_CORES = 8
GROUPS = [[0, 1, 2, 3], [4, 5, 6, 7]]
NCHUNK = 4           # seq chunks per batch
SPC = T // NCHUNK    # 32 steps per core
NTOK = ONE * SPC     # 384 tokens per core
NDEC = 3 * SPC       # 96 decoder tokens per core

DT = mybir.dt.bfloat16
F32 = mybir.dt.float32
F32R = mybir.dt.float32r
BF = ml_dtypes.bfloat16

EPS = 1e-5

# ---------------- environment workarounds ----------------
try:
    tile_utils.max_sbuf_usage = 207 * 1024
except Exception:
    pass

# This walrus build rejects >1 sync wait on the kernel-tail Drain; split the
# waits across extra SP nops.
_MAXW = 1


def _patched_drain_and_barrier(self, tick_clock, wait_clock):
    nc = self.nc
    drain_inst = nc.sync.drain()
    wait_clock.add_sem_waits(
        drain_inst.ins, ScopedClock({None: tick_clock.global_clock})
    )
    si = drain_inst.ins.sync_info
    waits = list(si.on_wait) if si is not None else []
    if len(waits) > _MAXW:
        drain_inst.ins.sync_info = bass_rust.SyncInfo(
            on_wait=waits[:_MAXW], on_update=list(si.on_update)
        )
        rest = waits[_MAXW:]
        for i in range(0, len(rest), _MAXW):
            nop = nc.sync.nop(nofuse=True)
            nop.ins.sync_info = bass_rust.SyncInfo(
                on_wait=rest[i : i + _MAXW], on_update=[]
            )
    nc.all_engine_barrier()
    assert self.sems is not None
    popped = nc._tile_sem_poison_stack.pop()
    assert popped is self._sem_poison
    nc.clear_and_free_semaphores(list(self.sems.allocated().values()))
    nc.all_engine_barrier()


TileContext._drain_and_barrier = _patched_drain_and_barrier


def _split_sync_waits(nc, maxw=1):
    """This walrus build rejects instructions with more than one sync wait;
    hoist excess semaphore waits onto same-engine nop instructions placed
    immediately before the owning instruction."""
    counter = 0
    for bb in nc.main_func.blocks:
        new_list = []
        for inst in bb.instructions:
            si = inst.sync_info
            if si is not None and len(si.on_wait) > maxw:
                sems = [w for w in si.on_wait if w.sync_type == "semaphore"]
                other = [w for w in si.on_wait if w.sync_type != "semaphore"]
                nkeep = max(0, maxw - len(other))
                keep, spill = sems[:nkeep], sems[nkeep:]
                for i in range(0, len(spill), maxw):
                    counter += 1
                    nop = mybir.InstNoOp(
                        name=f"wsplit_{counter}",
                        sync_info=bass_rust.SyncInfo(
                            on_wait=spill[i : i + maxw], on_update=[]
                        ),
                        bass_nofuse=True,
                        engine=inst.engine,
                    )
                    nc.register_instruction(nop, overwrite=True)
                    new_list.append(nop)
                if spill:
                    inst.sync_info = bass_rust.SyncInfo(
                        on_wait=other + keep, on_update=list(si.on_update)
                    )
            new_list.append(inst)
        bb.instructions = new_list
    return counter


# ---------------- host-side prep ----------------

def _block_mask_np():
    q = np.arange(S)[:, None]
    k = np.arange(S)[None, :]
    qp, kp = q % ONE, k % ONE
    qs, ks = q // ONE, k // ONE
    q_ctx, q_ao, q_ra = qp < TTA, qp == TTA, qp > TTA
    k_ctx, k_ao = kp < TTA, kp == TTA
    past, same = ks < qs, ks == qs
    in_hist = (qs - ks) <= T
    return (
        (past & (~k_ao) & in_hist)
        | (same & q_ctx & k_ctx)
        | (same & q_ao & (k_ctx | k_ao))
        | (same & q_ra & (~k_ao))
    )


def _colblock(w, bf):
    """[nl, K, M] -> [nl, M/128, K/128, 128, 128] bf16 column-block layout:
    out[i, m, k, p, c] = w[i, k*128+p, m*128+c]."""
    nl, K, M = w.shape
    r = w.reshape(nl, K // 128, 128, M // 128, 128)
    return bf(np.transpose(r, (0, 3, 1, 2, 4)))


def _prep_host(inputs):
    """Build shared weight arrays + per-core input arrays (numpy)."""
    f32 = lambda x: np.ascontiguousarray(np.asarray(x, dtype=np.float32))
    bf = lambda x: np.ascontiguousarray(np.asarray(x, dtype=np.float32).astype(BF))

    shared = {}
    shared["w_ram"] = bf(inputs["W_ram"])

    wqkv = f32(inputs["Wqkv"]).copy()
    wqkv[:, :, :D] *= 1.0 / np.sqrt(HD)
    shared["wqkvr"] = _colblock(wqkv, bf)
    shared["wor"] = _colblock(f32(inputs["Wo"]), bf)
    shared["wm1r"] = _colblock(f32(inputs["Wm1"]), bf)
    shared["wm2r"] = _colblock(f32(inputs["Wm2"]), bf)

    dwqkv = f32(inputs["dWqkv"]).copy()
    dwqkv[:, :, :D] *= 1.0 / np.sqrt(DHD)
    shared["dwqkvr"] = _colblock(dwqkv, bf)
    shared["dwor"] = _colblock(f32(inputs["dWo"]), bf)
    shared["dwm1r"] = _colblock(f32(inputs["dWm1"]), bf)
    shared["dwm2r"] = _colblock(f32(inputs["dWm2"]), bf)

    ln_affines = {}
    for name in ("ln1", "ln2", "dln1", "dln2"):
        ln_affines[name] = (f32(inputs[name + "_w"]), f32(inputs[name + "_b"]))
    for name in ("lnf", "dlnf"):
        ln_affines[name] = (
            f32(inputs[name + "_w"])[None, :],
            f32(inputs[name + "_b"])[None, :],
        )

    shared["ident"] = np.eye(128, dtype=np.float32).astype(BF)
    e16 = np.zeros((16, D), dtype=np.float32)
    for h in range(16):
        e16[h, h * 64 : (h + 1) * 64] = 1.0
    shared["e16"] = e16
    e8 = np.zeros((8, D), dtype=np.float32)
    for h in range(8):
        e8[h, h * 128 : (h + 1) * 128] = 1.0
    shared["e8"] = e8
    shared["ones_row"] = np.ones((1, 128), dtype=np.float32)

    tmpl_step = np.zeros((ONE, D), dtype=np.float32)
    tmpl_step[:N_IMG] = f32(inputs["img_pos"])[0]
    tmpl_step[N_IMG] = f32(inputs["think_pos"])[0, 0]
    tmpl_step[TTA] = f32(inputs["action_out_token"])[0, 0]
    tmpl_step[TTA + 1 :] = f32(inputs["action_pos"])[0]
    template = np.tile(tmpl_step, (SPC, 1)).T  # [D, 384]
    shared["template"] = np.ascontiguousarray(template)

    bm = _block_mask_np()
    ram = f32(inputs["ram"])
    actin = f32(inputs["action_embeddings_in"])

    per_core = []
    for core in range(N_CORES):
        b, c = divmod(core, NCHUNK)
        s0 = c * SPC
        cm = {}
        r = ram[b, s0 : s0 + SPC].reshape(SPC, N_IMG, CH)
        cm["ramT"] = np.ascontiguousarray(r.reshape(SPC * N_IMG, CH).T.astype(BF))
        a = actin[b, s0 : s0 + SPC]
        cm["actinT"] = np.ascontiguousarray(a.reshape(SPC * N_ACT, D).T)
        q_lo = s0 * ONE
        cm["maskT"] = np.ascontiguousarray(
            bm[q_lo : q_lo + NTOK, :].T.astype(np.float32).astype(BF)
        )
        per_core.append(cm)

    dm = np.zeros((NDEC, NDEC), dtype=np.float32)
    for s in range(SPC):
        dm[3 * s : 3 * s + 3, 3 * s : 3 * s + 3] = 1.0
    shared["dmaskT"] = dm.astype(BF)

    return shared, per_core, ln_affines


# ---------------- device kernel builder ----------------


def _affine_trivial(wb):
    w, b = wb
    return bool(np.all(w == 1.0) and np.all(b == 0.0))


def _build_nc(ln_affines):
    nc = bass.Bass()

    P = {}
    decl = nc.declare_dram_parameter
    P["ramT"] = decl("ramT", [CH, SPC * N_IMG], DT, isOutput=False)
    P["actinT"] = decl("actinT", [D, SPC * N_ACT], F32, isOutput=False)
    P["maskT"] = decl("maskT", [S, NTOK], DT, isOutput=False)
    P["dmaskT"] = decl("dmaskT", [NDEC, NDEC], DT, isOutput=False)
    P["template"] = decl("template", [D, NTOK], F32, isOutput=False)
    P["w_ram"] = decl("w_ram", [CH, D], DT, isOutput=False)
    P["wqkvr"] = decl("wqkvr", [L, 24, 8, 128, 128], DT, isOutput=False)
    P["wor"] = decl("wor", [L, 8, 8, 128, 128], DT, isOutput=False)
    P["wm1r"] = decl("wm1r", [L, 32, 8, 128, 128], DT, isOutput=False)
    P["wm2r"] = decl("wm2r", [L, 8, 32, 128, 128], DT, isOutput=False)
    P["dwqkvr"] = decl("dwqkvr", [LD, 24, 8, 128, 128], DT, isOutput=False)
    P["dwor"] = decl("dwor", [LD, 8, 8, 128, 128], DT, isOutput=False)
    P["dwm1r"] = decl("dwm1r", [LD, 32, 8, 128, 128], DT, isOutput=False)
    P["dwm2r"] = decl("dwm2r", [LD, 8, 32, 128, 128], DT, isOutput=False)
    P["ident"] = decl("ident", [128, 128], DT, isOutput=False)
    P["e16"] = decl("e16", [16, D], F32R, isOutput=False)
    P["e8"] = decl("e8", [8, D], F32R, isOutput=False)
    P["ones_row"] = decl("ones_row", [1, 128], F32R, isOutput=False)
    for name, wb in ln_affines.items():
        if not _affine_trivial(wb):
            nl = 1 if name in ("lnf", "dlnf") else (LD if name.startswith("d") else L)
            P[name + "_w"] = decl(name + "_w", [nl, D], F32, isOutput=False)
            P[name + "_b"] = decl(name + "_b", [nl, D], F32, isOutput=False)
    y_out = decl("y", [D, NDEC], F32, isOutput=True)

    with ExitStack() as ctx:
        tc = ctx.enter_context(TileContext(nc))

        const = ctx.enter_context(tc.tile_pool(name="const", bufs=1))
        xpool = ctx.enter_context(tc.tile_pool(name="xpool", bufs=1))
        sq_p = ctx.enter_context(tc.tile_pool(name="sq", bufs=3))
        xhb_p = ctx.enter_context(tc.tile_pool(name="xhb", bufs=9))
        qb_p = ctx.enter_context(tc.tile_pool(name="qb", bufs=9))
        kvt_p = ctx.enter_context(tc.tile_pool(name="kvt", bufs=6))
        dq_p = ctx.enter_context(tc.tile_pool(name="dq", bufs=17))
        pb_p = ctx.enter_context(tc.tile_pool(name="pb", bufs=13))
        kth_p = ctx.enter_context(tc.tile_pool(name="kth", bufs=3))
        vsb_p = ctx.enter_context(tc.tile_pool(name="vsb", bufs=12))
        vtm_p = ctx.enter_context(tc.tile_pool(name="vtm", bufs=4))
        dvtm_p = ctx.enter_context(tc.tile_pool(name="dvtm", bufs=2))
        au_p = ctx.enter_context(tc.tile_pool(name="au", bufs=9))
        atb_p = ctx.enter_context(tc.tile_pool(name="atb", bufs=9))
        hb_p = ctx.enter_context(tc.tile_pool(name="hb", bufs=17))
        w_p = ctx.enter_context(tc.tile_pool(name="w", bufs=3))
        wm2_p = ctx.enter_context(tc.tile_pool(name="wm2", bufs=2))
        st_p = ctx.enter_context(tc.tile_pool(name="st", bufs=2))
        st1_p = ctx.enter_context(tc.tile_pool(name="st1", bufs=1))

        ps_mm = ctx.enter_context(tc.tile_pool(name="ps_mm", bufs=2, space="PSUM"))
        ps_sc = ctx.enter_context(tc.tile_pool(name="ps_sc", bufs=2, space="PSUM"))
        ps_pv = ctx.enter_context(tc.tile_pool(name="ps_pv", bufs=2, space="PSUM"))
        ps_tr = ctx.enter_context(tc.tile_pool(name="ps_tr", bufs=2, space="PSUM"))
        ps_st = ps_tr
        dram = ctx.enter_context(tc.tile_pool(name="dram", bufs=2, space="DRAM"))

        # --- consts ---
        ident = const.tile([128, 128], DT, tag="ident")
        nc.sync.dma_start(ident[:], P["ident"][:])
        ones_row = const.tile([1, 128], F32R, tag="ones_row")
        nc.sync.dma_start(ones_row[:], P["ones_row"][:])
        ones_col_b = const.tile([128, 1], DT, tag="ones_col")
        nc.gpsimd.memset(ones_col_b[:], 1.0)
        eps_t = const.tile([1, 1], F32, tag="eps")
        nc.gpsimd.memset(eps_t[:], EPS)
        e16 = const.tile([16, D], F32R, tag="e16")
        nc.sync.dma_start(e16[:], P["e16"][:])
        e8 = const.tile([8, D], F32R, tag="e8")
        nc.sync.dma_start(e8[:], P["e8"][:])
        maskb = [
            const.tile([128, NTOK], DT, tag=f"mask{k}", name=f"mask{k}")
            for k in range(12)
        ]
        for k in range(12):
            nc.sync.dma_start(maskb[k][:], P["maskT"][k * 128 : (k + 1) * 128, :])
        dmask = const.tile([NDEC, NDEC], DT, tag="dmask")
        nc.sync.dma_start(dmask[:], P["dmaskT"][:])
        actin_sb = [
            const.tile([128, SPC * N_ACT], F32, tag=f"actin{m}", name=f"actin{m}")
            for m in range(8)
        ]
        for m in range(8):
            nc.sync.dma_start(actin_sb[m][:], P["actinT"][m * 128 : (m + 1) * 128, :])

        ln_aff_sb = {}
        for name, wb in ln_affines.items():
            if not _affine_trivial(wb):
                nl = 1 if name in ("lnf", "dlnf") else (LD if name.startswith("d") else L)
                wt = const.tile([128, 8 * nl], F32, tag=f"aff_w_{name}")
                bt = const.tile([128, 8 * nl], F32, tag=f"aff_b_{name}")
                nc.sync.dma_start(
                    wt[:], P[name + "_w"][:].rearrange("l (m p) -> p (l m)", p=128)
                )
                nc.sync.dma_start(
                    bt[:], P[name + "_b"][:].rearrange("l (m p) -> p (l m)", p=128)
                )
                ln_aff_sb[name] = (wt, bt)

        # --- helpers ---
        def ln_stats(x_tiles, ntok, nk):
            s1 = ps_st.tile([1, ntok], F32, tag="aux", name="s1")
            s2 = ps_st.tile([1, ntok], F32, tag="aux", name="s2")
            for k in range(nk):
                xb = sq_p.tile([128, ntok], DT, tag="sq", name="xb")
                nc.vector.tensor_copy(xb[:], x_tiles[k][:])
                nc.tensor.matmul(
                    s1[:], ones_col_b[:], xb[:],
                    start=(k == 0), stop=(k == nk - 1), skip_group_check=True,
                )
                sq = sq_p.tile([128, ntok], DT, tag="sq", name="sq")
                nc.vector.tensor_tensor(
                    sq[:], xb[:], xb[:], mybir.AluOpType.mult
                )
                nc.tensor.matmul(
                    s2[:], ones_col_b[:], sq[:],
                    start=(k == 0), stop=(k == nk - 1), skip_group_check=True,
                )
            negm = st1_p.tile([1, ntok], F32, tag="negm", name="negm")
            nc.vector.tensor_scalar_mul(negm[:], s1[:], -1.0 / (nk * 128))
            # t2 carries e2 -> var -> ln(var+eps) -> r in place
            t2 = st1_p.tile([1, ntok], F32, tag="t2", name="t2")
            nc.vector.tensor_scalar_mul(t2[:], s2[:], 1.0 / (nk * 128))
            m2 = st1_p.tile([1, ntok], F32, tag="m2", name="m2")
            nc.vector.tensor_tensor(m2[:], negm[:], negm[:], mybir.AluOpType.mult)
            nc.vector.tensor_tensor(t2[:], t2[:], m2[:], mybir.AluOpType.subtract)
            # rsqrt(v + eps) = exp(-0.5 * ln(v + eps)); ln/exp share one ACT
            # table set with the attention exp, avoiding table swaps.
            nc.scalar.activation(
                t2[:], t2[:], mybir.ActivationFunctionType.Ln,
                bias=eps_t[:], scale=1.0,
            )
            r = t2
            nc.scalar.activation(
                r[:], t2[:], mybir.ActivationFunctionType.Exp,
                bias=0.0, scale=-0.5,
            )
            negmr = m2
            nc.vector.tensor_tensor(negmr[:], negm[:], r[:], mybir.AluOpType.mult)
            r_r = st1_p.tile([1, ntok], F32R, tag="r_r", name="r_r")
            nc.vector.tensor_copy(r_r[:], r[:])
            negmr_r = st1_p.tile([1, ntok], F32R, tag="negmr_r", name="negmr_r")
            nc.vector.tensor_copy(negmr_r[:], negmr[:])
            psR = ps_st.tile([128, ntok], F32, tag="aux", name="psR")
            nc.tensor.matmul(psR[:], ones_row[:], r_r[:])
            psC = ps_st.tile([128, ntok], F32, tag="aux", name="psC")
            nc.tensor.matmul(psC[:], ones_row[:], negmr_r[:])
            Rsb = st_p.tile([128, ntok], F32, tag="Rsb", name="Rsb")
            nc.vector.tensor_copy(Rsb[:], psR[:])
            Csb = st_p.tile([128, ntok], F32, tag="Csb", name="Csb")
            nc.vector.tensor_copy(Csb[:], psC[:])
            return Rsb, Csb

        def ln_materialize(x_tiles, ntok, nk, aff, out_pool, out_tag, out_dtype=DT):
            Rsb, Csb = ln_stats(x_tiles, ntok, nk)
            outs = []
            for k in range(nk):
                t = out_pool.tile([128, ntok], out_dtype, tag=out_tag, name=out_tag)
                tmp = st_p.tile([128, ntok], F32, tag="lnm_tmp", name="lnm_tmp")
                nc.vector.tensor_tensor(
                    tmp[:], x_tiles[k][:], Rsb[:], mybir.AluOpType.mult
                )
                if aff is None:
                    nc.vector.tensor_tensor(t[:], tmp[:], Csb[:], mybir.AluOpType.add)
                else:
                    wt, bt, col = aff
                    nc.vector.tensor_tensor(
                        tmp[:], tmp[:], Csb[:], mybir.AluOpType.add
                    )
                    nc.vector.tensor_scalar(
                        t[:], tmp[:],
                        wt[:, col + k : col + k + 1],
                        bt[:, col + k : col + k + 1],
                        mybir.AluOpType.mult, mybir.AluOpType.add,
                    )
                outs.append(t)
            return outs

        def aff_of(name, layer_idx):
            if name in ln_aff_sb:
                wt, bt = ln_aff_sb[name]
                return (wt, bt, layer_idx * 8)
            return None

        def mm_colblock(wparam, li, m, xin, ntok, nk, wtag="w", wpool=None,
                        ps_tag="mm"):
            wpool = wpool or w_p
            wt = wpool.tile([128, nk * 128], DT, tag=wtag, name=wtag)
            nc.sync.dma_start(
                wt[:].rearrange("p (k c) -> p k c", c=128),
                wparam[li, m].rearrange("k p c -> p k c"),
            )
            ps = ps_mm.tile([128, ntok], F32, tag=ps_tag, name=ps_tag)
            for k in range(nk):
                nc.tensor.matmul(
                    ps[:], wt[:, k * 128 : (k + 1) * 128], xin[k][:],
                    start=(k == 0), stop=(k == nk - 1),
                )
            return ps

        # --- embedding ---
        X = [xpool.tile([128, NTOK], F32, tag=f"x{m}", name=f"x{m}") for m in range(8)]
        for m in range(8):
            nc.sync.dma_start(X[m][:], P["template"][m * 128 : (m + 1) * 128, :])

        ram_sb = [
            const.tile([128, SPC * N_IMG], DT, tag=f"ram{k}", name=f"ram{k}")
            for k in range(2)
        ]
        wram_sb = [
            const.tile([128, D], DT, tag=f"wram{k}", name=f"wram{k}")
            for k in range(2)
        ]
        for k in range(2):
            nc.sync.dma_start(ram_sb[k][:], P["ramT"][k * 128 : (k + 1) * 128, :])
            nc.sync.dma_start(wram_sb[k][:], P["w_ram"][k * 128 : (k + 1) * 128, :])
        for m in range(8):
            ps = ps_mm.tile([128, SPC * N_IMG], F32, tag="mm", name="img")
            for k in range(2):
                nc.tensor.matmul(
                    ps[:], wram_sb[k][:, m * 128 : (m + 1) * 128], ram_sb[k][:],
                    start=(k == 0), stop=(k == 1),
                )
            xv = X[m][:].rearrange("p (s o) -> p s o", o=ONE)[:, :, 0:N_IMG]
            iv = ps[:].rearrange("p (s j) -> p s j", j=N_IMG)
            nc.vector.tensor_tensor(xv, xv, iv, mybir.AluOpType.add)
            xa = X[m][:].rearrange("p (s o) -> p s o", o=ONE)[
                :, :, TTA + 1 : TTA + 1 + N_ACT
            ]
            av = actin_sb[m][:].rearrange("p (s j) -> p s j", j=N_ACT)
            nc.vector.tensor_tensor(xa, xa, av, mybir.AluOpType.add)

        # --- backbone ---
        for li in range(L):
            xhb = ln_materialize(X, NTOK, 8, aff_of("ln1", li), xhb_p, "xhb")

            # qkv: out tiles 0..7 = Q (kept), 8..15 = K (DMA out at once),
            # 16..23 = V (transposed inline)
            KSZ = 8 * 128 * NTOK
            kv_in = dram.tile([2 * KSZ], DT, tag="kv_in", name="kv_in")
            kin = kv_in[0:KSZ].rearrange("(a p f) -> a p f", p=128, f=NTOK)
            vin = kv_in[KSZ : 2 * KSZ].rearrange("(a p f) -> a p f", p=128, f=D)
            qkvb = [None] * 8
            vtm = [
                vtm_p.tile([128, D], DT, tag="vtm", name="vtm") for _ in range(3)
            ]
            for m in range(24):
                ps = mm_colblock(P["wqkvr"], li, m, xhb, NTOK, 8, wtag="wq")
                if m < 8:
                    t = qb_p.tile([128, NTOK], DT, tag="qb", name="qb")
                    nc.any.tensor_copy(out=t[:], in_=ps[:])
                    qkvb[m] = t
                else:
                    t = kvt_p.tile([128, NTOK], DT, tag="kvt", name="kvt")
                    nc.any.tensor_copy(out=t[:], in_=ps[:])
                    if m < 16:
                        nc.sync.dma_start(kin[m - 8], t[:])
                    else:
                        dv = m - 16
                        for t3 in range(3):
                            pst = ps_tr.tile([128, 128], DT, tag="aux", name="tr")
                            nc.tensor.transpose(
                                pst[:], t[:, t3 * 128 : (t3 + 1) * 128], ident[:]
                            )
                            nc.any.tensor_copy(
                                out=vtm[t3][:, dv * 128 : (dv + 1) * 128], in_=pst[:]
                            )
            for t3 in range(3):
                nc.sync.dma_start(vin[t3], vtm[t3][:])
            kv_all = dram.tile([NCHUNK * 2 * KSZ], DT, tag="kv_all", name="kv_all")
            nc.gpsimd.collective_compute(
                "AllGather",
                mybir.AluOpType.bypass,
                replica_groups=GROUPS,
                ins=[kv_in[:].opt()],
                outs=[kv_all[:].opt()],
            )
            vsb = [
                vsb_p.tile([128, 16 * 65], DT, tag="vsb", name="vsb")
                for _ in range(12)
            ]
            for g in range(NCHUNK):
                base = g * 2 * KSZ
                vg = kv_all[base + KSZ : base + 2 * KSZ].rearrange(
                    "(a p f) -> a p f", p=128, f=D
                )
                for t3 in range(3):
                    kc = g * 3 + t3
                    dst = vsb[kc][:].rearrange("p (h e) -> p h e", e=65)[:, :, 0:64]
                    nc.sync.dma_start(dst, vg[t3].rearrange("p (h e) -> p h e", e=64))
            for kc in range(12):
                ones_v = vsb[kc][:].rearrange("p (h e) -> p h e", e=65)[:, :, 64:65]
                nc.gpsimd.memset(ones_v, 1.0)

            # attention
            rall = st_p.tile([16, NTOK], F32, tag="rall", name="rall")
            attnU = [
                au_p.tile([128, NTOK], DT, tag="au", name="au") for _ in range(8)
            ]
            kth = None
            for h in range(16):
                if h % 2 == 0:
                    kth = kth_p.tile([128, S], DT, tag="kth", name="kth")
                    for g in range(NCHUNK):
                        base = g * 2 * KSZ
                        kg = kv_all[base : base + KSZ].rearrange(
                            "(a p f) -> a p f", p=128, f=NTOK
                        )
                        nc.sync.dma_start(
                            kth[:, g * NTOK : (g + 1) * NTOK], kg[h // 2]
                        )
                lo = (h % 2) * 64
                qh = qkvb[h // 2][lo : lo + 64, :]
                pbs = []
                for kc in range(12):
                    ps_s = ps_sc.tile([128, NTOK], F32, tag="score", name="score")
                    nc.tensor.matmul(
                        ps_s[:], kth[lo : lo + 64, kc * 128 : (kc + 1) * 128], qh
                    )
                    pb = pb_p.tile([128, NTOK], DT, tag="pb", name="pb")
                    nc.scalar.activation(
                        pb[:], ps_s[:], mybir.ActivationFunctionType.Exp
                    )
                    nc.vector.tensor_tensor(
                        pb[:], pb[:], maskb[kc][:], mybir.AluOpType.mult
                    )
                    pbs.append(pb)
                ps_o = ps_pv.tile([65, NTOK], F32, tag="pv", name="pv")
                for kc in range(12):
                    nc.tensor.matmul(
                        ps_o[:], vsb[kc][:, h * 65 : h * 65 + 65], pbs[kc][:],
                        start=(kc == 0), stop=(kc == 11),
                    )
                rstage = st_p.tile([65, NTOK], F32, tag="rstage", name="rstage")
                nc.vector.tensor_copy(rstage[64:65, :], ps_o[64:65, :])
                nc.sync.dma_start(rall[h : h + 1, :], rstage[64:65, :])
                nc.any.tensor_copy(
                    out=attnU[h // 2][(h % 2) * 64 : (h % 2) * 64 + 64, :],
                    in_=ps_o[0:64, :],
                )
            nc.vector.reciprocal(rall[:], rall[:])
            rall_r = st_p.tile([16, NTOK], F32R, tag="rall_r", name="rall_r")
            nc.vector.tensor_copy(rall_r[:], rall[:])
            attnTb = []
            for m in range(8):
                ps_rb = ps_sc.tile([128, NTOK], F32, tag="score", name="rbc")
                nc.tensor.matmul(
                    ps_rb[:],
                    e16[:, m * 128 : (m + 1) * 128],
                    rall_r[:],
                )
                t = atb_p.tile([128, NTOK], DT, tag="atb", name="atb")
                nc.vector.tensor_tensor(
                    t[:], attnU[m][:], ps_rb[:], mybir.AluOpType.mult
                )
                attnTb.append(t)

            # Wo + residual
            for m in range(8):
                ps = mm_colblock(P["wor"], li, m, attnTb, NTOK, 8, wtag="wo")
                nc.vector.tensor_tensor(X[m][:], X[m][:], ps[:], mybir.AluOpType.add)

            # MLP
            xh2b = ln_materialize(X, NTOK, 8, aff_of("ln2", li), xhb_p, "xhb")
            for half in range(2):
                hb = []
                for mh in range(16):
                    m = half * 16 + mh
                    ps = mm_colblock(P["wm1r"], li, m, xh2b, NTOK, 8, wtag="wm1")
                    t = hb_p.tile([128, NTOK], DT, tag="hb", name="hb")
                    nc.scalar.activation(
                        t[:], ps[:], mybir.ActivationFunctionType.Gelu_apprx_tanh
                    )
                    hb.append(t)
                for m in range(8):
                    wt = wm2_p.tile([128, 16 * 128], DT, tag="wm2", name="wm2")
                    nc.sync.dma_start(
                        wt[:].rearrange("p (k c) -> p k c", c=128),
                        P["wm2r"][li, m, half * 16 : half * 16 + 16].rearrange(
                            "k p c -> p k c"
                        ),
                    )
                    ps = ps_mm.tile([128, NTOK], F32, tag="mm", name="mm")
                    for k in range(16):
                        nc.tensor.matmul(
                            ps[:], wt[:, k * 128 : (k + 1) * 128], hb[k][:],
                            start=(k == 0), stop=(k == 15),
                        )
                    nc.vector.tensor_tensor(
                        X[m][:], X[m][:], ps[:], mybir.AluOpType.add
                    )

        # --- final LN on action_out tokens; build decoder stream ---
        xao_c = []
        for m in range(8):
            t = au_p.tile([128, SPC], F32, tag="xao", name="xao")
            nc.vector.tensor_copy(t[:], X[m][:, TTA::ONE])
            xao_c.append(t)
        zao = ln_materialize(xao_c, SPC, 8, aff_of("lnf", 0), au_p, "zao",
                             out_dtype=F32)
        Z = [
            xpool.tile([128, NDEC], F32, tag=f"z{m}", name=f"z{m}") for m in range(8)
        ]
        for m in range(8):
            zv = Z[m][:].rearrange("p (s t) -> p s t", t=3)
            nc.vector.tensor_copy(zv[:, :, 0:1], zao[m][:, :, None])
            nc.vector.tensor_copy(
                zv[:, :, 1:3], actin_sb[m][:].rearrange("p (s j) -> p s j", j=2)
            )

        # --- decoder ---
        for li in range(LD):
            zhb = ln_materialize(Z, NDEC, 8, aff_of("dln1", li), xhb_p, "xhb")
            qkvb = [None] * 16
            vtm = dvtm_p.tile([NDEC, D], DT, tag="dvtm", name="dvtm")
            for m in range(24):
                ps = mm_colblock(P["dwqkvr"], li, m, zhb, NDEC, 8, wtag="wq")
                if m < 16:
                    t = dq_p.tile([128, NDEC], DT, tag="dqk", name="dqk")
                    nc.any.tensor_copy(out=t[:], in_=ps[:])
                    qkvb[m] = t
                else:
                    t = kvt_p.tile([128, NDEC], DT, tag="kvt", name="dkvt")
                    nc.any.tensor_copy(out=t[:], in_=ps[:])
                    dv = m - 16
                    pst = ps_tr.tile([NDEC, 128], DT, tag="aux", name="dtr")
                    nc.tensor.transpose(pst[:], t[:, :], ident[:])
                    nc.any.tensor_copy(
                        out=vtm[:, dv * 128 : (dv + 1) * 128], in_=pst[:]
                    )
            rall = st_p.tile([8, NDEC], F32, tag="drall", name="drall")
            attnU = [
                au_p.tile([128, NDEC], DT, tag="au", name="dau") for _ in range(8)
            ]
            for h in range(8):
                ps_s = ps_sc.tile([NDEC, NDEC], F32, tag="score", name="dscore")
                nc.tensor.matmul(ps_s[:], qkvb[8 + h][:, :], qkvb[h][:, :])
                pb = pb_p.tile([NDEC, NDEC], DT, tag="pb", name="dpb")
                nc.scalar.activation(pb[:], ps_s[:], mybir.ActivationFunctionType.Exp)
                nc.vector.tensor_tensor(pb[:], pb[:], dmask[:], mybir.AluOpType.mult)
                ps_o = ps_pv.tile([128, NDEC], F32, tag="pv", name="dpv")
                nc.tensor.matmul(ps_o[:], vtm[:, h * 128 : (h + 1) * 128], pb[:])
                ps_r = ps_pv.tile([1, NDEC], F32, tag="pv", name="dpvr")
                nc.tensor.matmul(ps_r[:], ones_col_b[:NDEC, :], pb[:])
                rstage = st_p.tile([1, NDEC], F32, tag="drstage", name="drstage")
                nc.vector.tensor_copy(rstage[:], ps_r[:])
                nc.sync.dma_start(rall[h : h + 1, :], rstage[:])
                nc.any.tensor_copy(out=attnU[h][:], in_=ps_o[:])
            nc.vector.reciprocal(rall[:], rall[:])
            rall_r = st_p.tile([8, NDEC], F32R, tag="drall_r", name="drall_r")
            nc.vector.tensor_copy(rall_r[:], rall[:])
            attnTb = []
            for m in range(8):
                ps_rb = ps_sc.tile([128, NDEC], F32, tag="score", name="drbc")
                nc.tensor.matmul(
                    ps_rb[:],
                    e8[:, m * 128 : (m + 1) * 128],
                    rall_r[:],
                )
                t = atb_p.tile([128, NDEC], DT, tag="atb", name="datb")
                nc.vector.tensor_tensor(
                    t[:], attnU[m][:], ps_rb[:], mybir.AluOpType.mult
                )
                attnTb.append(t)
            for m in range(8):
                ps = mm_colblock(P["dwor"], li, m, attnTb, NDEC, 8, wtag="wo")
                nc.vector.tensor_tensor(Z[m][:], Z[m][:], ps[:], mybir.AluOpType.add)

            zh2b = ln_materialize(Z, NDEC, 8, aff_of("dln2", li), xhb_p, "xhb")
            for half in range(2):
                hb = []
                for mh in range(16):
                    m = half * 16 + mh
                    ps = mm_colblock(P["dwm1r"], li, m, zh2b, NDEC, 8, wtag="wm1")
                    t = hb_p.tile([128, NDEC], DT, tag="hb", name="dhb")
                    nc.scalar.activation(
                        t[:], ps[:], mybir.ActivationFunctionType.Gelu_apprx_tanh
                    )
                    hb.append(t)
                for m in range(8):
                    wt = wm2_p.tile([128, 16 * 128], DT, tag="wm2", name="dwm2")
                    nc.sync.dma_start(
                        wt[:].rearrange("p (k c) -> p k c", c=128),
                        P["dwm2r"][li, m, half * 16 : half * 16 + 16].rearrange(
                            "k p c -> p k c"
                        ),
                    )
                    ps = ps_mm.tile([128, NDEC], F32, tag="mm", name="dmm")
                    for k in range(16):
                        nc.tensor.matmul(
                            ps[:], wt[:, k * 128 : (k + 1) * 128], hb[k][:],
                            start=(k == 0), stop=(k == 15),
                        )
                    nc.vector.tensor_tensor(
                        Z[m][:], Z[m][:], ps[:], mybir.AluOpType.add
                    )

        zf = ln_materialize(Z, NDEC, 8, aff_of("dlnf", 0), au_p, "zf", out_dtype=F32)
        for m in range(8):
            nc.sync.dma_start(y_out[m * 128 : (m + 1) * 128, :], zf[m][:])

    _split_sync_waits(nc)
    return nc


# ---------------- public entry point ----------------

_CACHE = {}
LAST_RESULT = None


def kernel(**inputs):
    shared, per_core, ln_affines = _prep_host(inputs)

    key = tuple(sorted((n, _affine_trivial(wb)) for n, wb in ln_affines.items()))
    if key not in _CACHE:
        _CACHE[key] = _build_nc(ln_affines)
    nc = _CACHE[key]

    in_maps = []
    for core in range(N_CORES):
        m = dict(per_core[core])
        for k in (
            "template", "dmaskT", "w_ram", "wqkvr", "wor", "wm1r", "wm2r",
            "dwqkvr", "dwor", "dwm1r", "dwm2r", "ident", "e16", "e8", "ones_row",
        ):
            m[k] = shared[k]
        for name, wb in ln_affines.items():
            if not _affine_trivial(wb):
                w, b = wb
                nl = 1 if name in ("lnf", "dlnf") else (LD if name.startswith("d") else L)
                m[name + "_w"] = np.ascontiguousarray(w.reshape(nl, D))
                m[name + "_b"] = np.ascontiguousarray(b.reshape(nl, D))
        in_maps.append(m)

    import os

    trace = os.environ.get("BASS_KERNEL_TRACE", "") == "1"
    res = run_bass_kernel_spmd(
        nc, in_maps, core_ids=list(range(N_CORES)), trace=trace
    )
    global LAST_RESULT
    LAST_RESULT = res

    out = np.zeros((B, T, N_ACT + 1, D), dtype=np.float32)
    for core in range(N_CORES):
        b, c = divmod(core, NCHUNK)
        y = res.results[core]["y"]  # [D, 96]
        out[b, c * SPC : (c + 1) * SPC] = np.asarray(y, dtype=np.float32).T.reshape(
            SPC, N_ACT + 1, D
        )
    return out


if __name__ == "__main__":
    sys.path.insert(0, "/root/problem")
    import reference

    inputs = {k: np.asarray(v) for k, v in reference.setup_inputs().items()}
    out = kernel(**inputs)
    print("out shape", out.shape)
